# revision 40
# baseline (speedup 1.0000x reference)
"""Causal self-attention (B=2, T=4096, C=768, H=12, D=64) on 8 TRN2 NeuronCores.

Sharding: tensor-parallel over heads x data-parallel over batch.
  core i (i in 0..7): batch b = i // 4, heads hs..hs+2 where hs = 3 * (i % 4).

Per-core kernel (all matmuls bf16 with f32 PSUM accumulation), fully
software-pipelined per 512-token group tg so the Act engine (exp) starts
working from the first group instead of after the whole QKV phase:

  for tg in 0..7:
    1. x tiles [4x128, 768] DMA (fp32) -> DVE cast bf16 -> PE-transpose
       -> xT slices for this token group
    2. QKV^T projection for this group's 512 columns (5 PSUM tiles of
       q0|q1, k0|k1, q2|v2, k2|pad, v0|v1 rows)
    3. V' tiles for this k-group: PE-transposed V plus an appended ones
       column (the ones column accumulates softmax row sums during PV)
    4. partial out-projection of the PREVIOUS q-chunk (its OT settled
       during this group's QKV work, so the PE never waits on it)
    5. per-chunk ReduceScatter(add) of the previous chunk's rows across
       the 4-core group (8 small 0.79 MB collectives instead of 2 big
       ones; only the last one's latency is exposed)
    6. causal attention for q-chunk tg: S^T = K^T.T @ Q^T blocks packed
       into [128,1024] PSUM groups, exp with the 1/8 scale folded in (no
       max-subtraction; logits are O(1)), diagonal-block column
       truncation + triangular masking, PV' matmuls accumulating
       [65, 512] (row 64 = softmax denominators), then normalize via
       reciprocal + gpsimd partition-broadcast into OT
  tail: out-projection of chunk 7 + its ReduceScatter + output DMA.

PSUM budget (8 banks): xqps 2x1 (transposes+QKV) + sps 2x2 (S groups)
+ ops 2x1 (PV accum / out-proj).

Host side only shards/concatenates and pre-slices weight columns.
"""

import numpy as np

B, T, C, H, D = 2, 4096, 768, 12, 64
N_CORES = 8
HPC = 3            # heads per core
QCH = 512          # q chunk (free dim of S^T matmul)
KB = 128           # k block (partition dim of S^T)
NT = T // 128      # 32 row-tiles
NQC = T // QCH     # 8 q chunks
CCH = C // 128     # 6 contraction chunks
SCAP = 1024        # S-group PSUM capacity (2 banks)
PV8_QC = 4         # fp8 PV only for q-chunks >= this (early chunks have the
                   # smallest softmax support and thus the worst fp8 noise)
EXP_BIAS = -3.5    # constant logit shift: keeps exp within fp8e4m3 range
                   # (max observed logit ~8.5); cancels in the softmax
                   # normalization since all paths share it


def _build_nc(num_devices=N_CORES, replica_groups=None, dev_single=False,
              stop_after=None, xcast="vector", pbcopy="vector", reps=1,
              xmode="pe", pv8=True):
    import concourse.mybir as mybir
    import concourse.tile as tile
    from concourse import bacc

    if dev_single:
        num_devices = 1
    if replica_groups is None:
        replica_groups = [[0, 1, 2, 3], [4, 5, 6, 7]]

    fp32 = mybir.dt.float32
    bf16 = mybir.dt.bfloat16
    fp8 = mybir.dt.float8e4

    nc = bacc.Bacc("TRN2", target_bir_lowering=False, debug=False,
                   num_devices=num_devices)
    x_in = nc.dram_tensor("x", [T, C], fp32, kind="ExternalInput")
    wqkv_in = nc.dram_tensor("wqkv", [C, 640], fp32, kind="ExternalInput")
    wp_in = nc.dram_tensor("wp", [HPC * D, C], fp32, kind="ExternalInput")
    iden_in = nc.dram_tensor("iden", [128, 128], fp32, kind="ExternalInput")
    masks_in = nc.dram_tensor("masks", [128, 128], fp32, kind="ExternalInput")
    out = nc.dram_tensor("out", [T // 4, C], fp32, kind="ExternalOutput")

    q_loc = [(0, 0), (0, 64), (2, 0)]
    k_loc = [(1, 0), (1, 64), (3, 0)]
    v_loc = [(4, 0), (4, 64), (2, 64)]

    with tile.TileContext(nc) as tc:
        with tc.tile_pool(name="pers", bufs=1) as pers, \
             tc.tile_pool(name="dram", bufs=1, space="DRAM") as dram, \
             tc.tile_pool(name="xstage", bufs=4) as xstage, \
             tc.tile_pool(name="wstage", bufs=2) as wstage, \
             tc.tile_pool(name="xqps", bufs=2, space="PSUM") as xqps, \
             tc.tile_pool(name="sps", bufs=2, space="PSUM") as sps, \
             tc.tile_pool(name="ops", bufs=2, space="PSUM") as ops, \
             tc.tile_pool(name="ptp", bufs=8) as ptp, \
             tc.tile_pool(name="ystage", bufs=3) as ystage, \
             tc.tile_pool(name="epi", bufs=3) as epi:

            # ---- front DMAs: iden + first x group on the SP queue, weights
            # in parallel on the Activation HWDGE queue
            idf = xstage.tile([128, 128], fp32, tag="cst")
            nc.sync.dma_start(idf[:], iden_in.ap()[:])
            idb = pers.tile([128, 128], bf16)
            nc.vector.tensor_copy(idb[:], idf[:])

            xfs0 = []
            for ti in range(4 if xmode != "dmat" else 0):
                xf = xstage.tile([128, C], bf16, tag="xf")
                if xcast == "dma":
                    nc.gpsimd.dma_start(
                        xf[:], x_in.ap()[ti * 128:(ti + 1) * 128, :])
                else:
                    xff = xstage.tile([128, C], fp32, tag="xff")
                    nc.sync.dma_start(
                        xff[:], x_in.ap()[ti * 128:(ti + 1) * 128, :])
                    getattr(nc, xcast).tensor_copy(xf[:], xff[:])
                xfs0.append(xf)

            # ---- weights (overlap with tg=0 transposes)
            wqb = []
            for ci in range(CCH):
                wf = wstage.tile([128, 640], fp32, tag="wf")
                nc.scalar.dma_start(wf[:], wqkv_in.ap()[ci * 128:(ci + 1) * 128, :])
                wb = pers.tile([128, 640], bf16, name=f"wqb{ci}")
                nc.vector.tensor_copy(wb[:], wf[:])
                wqb.append(wb)
            wpf_a = wstage.tile([128, C], fp32, tag="wf")
            nc.scalar.dma_start(wpf_a[:], wp_in.ap()[0:128, :])
            wpb_a = pers.tile([128, C], bf16)
            nc.vector.tensor_copy(wpb_a[:], wpf_a[:])
            wpf_b = wstage.tile([64, C], fp32, tag="wf")
            nc.scalar.dma_start(wpf_b[:], wp_in.ap()[128:192, :])
            wpb_b = pers.tile([64, C], bf16)
            nc.vector.tensor_copy(wpb_b[:], wpf_b[:])
            mf = xstage.tile([128, 128], fp32, tag="cst")
            nc.scalar.dma_start(mf[:], masks_in.ap()[:, :])
            maskt = pers.tile([128, 128], bf16)
            nc.vector.tensor_copy(maskt[:], mf[:])

            # ---- persistent activations ----
            xT = [pers.tile([128, T], bf16, name=f"xT{ci}") for ci in range(CCH)]
            qkvT = [pers.tile([128, T], bf16, name=f"qkvT{m}") for m in range(5)]
            vpbuf = [pers.tile([128, NT * (D + 1)], bf16, name=f"vpbuf{h}")
                     for h in range(HPC)]
            vp = [[vpbuf[h][:, kt * (D + 1):(kt + 1) * (D + 1)]
                   for kt in range(NT)] for h in range(HPC)]
            vp8buf = [pers.tile([128, NT * D], fp8, name=f"vp8buf{h}")
                      for h in range(HPC)] if pv8 else None
            ebias = pers.tile([128, 1], fp32)
            nc.vector.memset(ebias[:], EXP_BIAS)
            if pv8:
                # DoubleRow rowsum stationary: M=32 (ISA minimum tile), ones
                # in output column 0 of each k-subtile, zeros elsewhere
                ones8 = pers.tile([128, 64], fp8)
                nc.vector.memset(ones8[:], 0.0)
                nc.vector.memset(ones8[:, 0:1], 1.0)
                nc.vector.memset(ones8[:, 32:33], 1.0)
            OT_a = pers.tile([128, T], bf16)   # heads 0,1 rows
            OT_b = pers.tile([64, T], bf16)    # head 2
            send = dram.tile([T, C], bf16)
            xb16 = dram.tile([T, C], bf16, name="xb16") if xmode == "dmat" \
                else None
            recvs = [dram.tile([QCH // 4, C], bf16, name=f"recv{c}")
                     for c in range(NQC)]

            def xprep(tg):
                """Cast-DMA x group tg to bf16 DRAM, then XBAR transpose-DMAs
                into the xT slices (replaces PE transposes entirely)."""
                rsl = slice(tg * QCH, (tg + 1) * QCH)
                nc.gpsimd.dma_start(xb16[rsl, :], x_in.ap()[rsl, :])
                for ci in range(CCH):
                    nc.scalar.dma_start_transpose(
                        xT[ci][:, rsl],
                        xb16[rsl, ci * 128:(ci + 1) * 128])

            def do_proj(c, half=None):
                """Partial out-projection of q-chunk c, then its RS.
                half=0/1 emits only that half-chunk (2 tts + half-RS)."""
                tts = range(4 * c, 4 * c + 4) if half is None else \
                    range(4 * c + 2 * half, 4 * c + 2 * half + 2)
                for tt in tts:
                    csl = slice(tt * 128, (tt + 1) * 128)
                    pA = ops.tile([128, 512], fp32, tag="op")
                    pB = ops.tile([128, 256], fp32, tag="op")
                    nc.tensor.matmul(pA[:], OT_a[:, csl], wpb_a[:, 0:512],
                                     start=True, stop=False)
                    nc.tensor.matmul(pA[:], OT_b[:, csl], wpb_b[:, 0:512],
                                     start=False, stop=True)
                    nc.tensor.matmul(pB[:], OT_a[:, csl], wpb_a[:, 512:768],
                                     start=True, stop=False)
                    nc.tensor.matmul(pB[:], OT_b[:, csl], wpb_b[:, 512:768],
                                     start=False, stop=True)
                    ysb = ystage.tile([128, C], bf16, tag="ysb")
                    nc.vector.tensor_copy(ysb[:, 0:512], pA[:])
                    getattr(nc, pbcopy).tensor_copy(ysb[:, 512:768], pB[:])
                    nc.sync.dma_start(send[csl, :], ysb[:])
                if half is None:
                    rlo, rhi, olo = c * QCH, (c + 1) * QCH, c * 128
                    rcv = recvs[c][:, :]
                else:
                    rlo = c * QCH + half * (QCH // 2)
                    rhi = rlo + QCH // 2
                    olo = c * 128 + half * 64
                    rcv = recvs[c][half * 64:half * 64 + 64, :]
                if dev_single:
                    nc.sync.dma_start(rcv, send[rlo:rlo + rcv.shape[0], :])
                else:
                    nc.gpsimd.collective_compute(
                        "ReduceScatter", mybir.AluOpType.add,
                        replica_groups=replica_groups,
                        ins=[send[rlo:rhi, :].opt()],
                        outs=[rcv.opt()])
                nc.gpsimd.dma_start(
                    out.ap()[olo:olo + rcv.shape[0], :], rcv)

            def emit_x_load(tg):
                xfs = []
                for ti in range(4 * tg, 4 * tg + 4):
                    xff = xstage.tile([128, C], fp32, tag="xff")
                    nc.sync.dma_start(
                        xff[:], x_in.ap()[ti * 128:(ti + 1) * 128, :])
                    xf = xstage.tile([128, C], bf16, tag="xf")
                    nc.vector.tensor_copy(xf[:], xff[:])
                    xfs.append(xf)
                return xfs

            def prep_units(tg, xfs):
                """Emission closures for x^T, QKV^T and V' of token group tg;
                interleaved into the PREVIOUS chunk's attention so the PE
                fills its slack while the Act engine streams exps."""
                tsl = slice(tg * QCH, (tg + 1) * QCH)
                units = []
                for ci in range(CCH):
                    def u_xt(ci=ci):
                        xps = xqps.tile([128, 512], bf16, tag="xq")
                        for j in range(4):
                            nc.tensor.transpose(
                                xps[:, j * 128:(j + 1) * 128],
                                xfs[j][:, ci * 128:(ci + 1) * 128], idb[:, :])
                        nc.vector.tensor_copy(xT[ci][:, tsl], xps[:])
                    units.append(u_xt)
                for m in (0, 2, 1, 3, 4):
                    def u_qkv(m=m):
                        ps = xqps.tile([128, QCH], fp32, tag="xq")
                        for ci in range(CCH):
                            nc.tensor.matmul(
                                ps[:],
                                wqb[ci][:, m * 128:(m + 1) * 128],
                                xT[ci][:, tsl],
                                start=(ci == 0), stop=(ci == CCH - 1),
                            )
                        nc.vector.tensor_copy(qkvT[m][:, tsl], ps[:])
                    units.append(u_qkv)
                for h in range(HPC):
                    def u_vp(h=h):
                        vm, vo = v_loc[h]
                        tp = xqps.tile([128, 4 * D], bf16, tag="xq")
                        for j in range(4):
                            kt = 4 * tg + j
                            nc.tensor.transpose(
                                tp[:, j * D:(j + 1) * D],
                                qkvT[vm][vo:vo + D, kt * 128:(kt + 1) * 128],
                                idb[vo:vo + D, vo:vo + D],
                            )
                        dst = vpbuf[h][:,
                                       4 * tg * (D + 1):(4 * tg + 4) * (D + 1)]
                        dst3 = dst.rearrange("p (g d) -> p g d", d=D + 1)
                        src3 = tp[:].rearrange("p (g d) -> p g d", d=D)
                        nc.vector.tensor_copy(dst3[:, :, 0:D], src3[:])
                        nc.vector.memset(dst3[:, :, D:D + 1], 1.0)
                        if pv8:
                            d8 = vp8buf[h][:, 4 * tg * D:(4 * tg + 4) * D]
                            nc.vector.tensor_copy(d8[:], tp[:])
                    units.append(u_vp)
                return units

            units = prep_units(0, xfs0)
            for u in units:
                u()

            for rep in range(reps):
              for tg in range(NQC):
                # prep work for the NEXT token group, paced into this
                # chunk's attention
                if tg + 1 < NQC or rep + 1 < reps:
                    xfs_n = emit_x_load((tg + 1) % NQC)
                    units = prep_units((tg + 1) % NQC, xfs_n)
                else:
                    units = []
                uptr = 0

                # ---- out-proj + RS of the previous chunk ----
                if tg >= 1:
                    do_proj(tg - 1)

                # ---- causal attention for q-chunk qc = tg ----
                qc = tg
                nkb = (qc + 1) * (QCH // KB)
                n_slots = HPC * (2 * qc + 2) if qc else HPC * 2
                slot = 0

                def after_group():
                    nonlocal uptr, slot
                    slot += 1
                    target = (len(units) * slot) // n_slots
                    while uptr < min(target, len(units)):
                        units[uptr]()
                        uptr += 1

                # per-kb: (kb, q_off, width): diag blocks (last 4) are
                # truncated to their causal column range [128m, 512).
                blocks = [(kb, 0, QCH) for kb in range(nkb - 4)]
                for m in (0, 3, 1, 2):
                    kb = nkb - 4 + m
                    blocks.append((kb, 128 * m, QCH - 128 * m))
                for h in range(HPC):
                    qm, qo = q_loc[h]
                    km, ko = k_loc[h]
                    use8 = pv8 and qc >= PV8_QC
                    op = ops.tile([D + 1, QCH], fp32, tag="op")
                    if use8:
                        rsum = ops.tile([32, QCH], fp32, tag="op", name="rsum")
                    else:
                        rsum = None
                    first_pv = True
                    gi = 0
                    if use8:
                        # non-diag full blocks in adjacent pairs: S pair-group
                        # -> one exp into fp8 pt -> one DoubleRow PV matmul
                        npair = (nkb - 4) // 2
                        for pi in range(npair):
                            kb0 = 2 * pi
                            sp = sps.tile([128, SCAP], fp32, tag="sp")
                            for j in range(2):
                                kb = kb0 + j
                                nc.tensor.matmul(
                                    sp[:, j * QCH:(j + 1) * QCH],
                                    qkvT[km][ko:ko + D, kb * KB:(kb + 1) * KB],
                                    qkvT[qm][qo:qo + D,
                                             qc * QCH:(qc + 1) * QCH],
                                    start=True, stop=True,
                                )
                            p8 = ptp.tile([128, SCAP], fp8, tag="p8")
                            nc.scalar.activation(
                                p8[:], sp[:],
                                mybir.ActivationFunctionType.Exp,
                                scale=0.125, bias=ebias[0:128, :])
                            p83 = p8[:].rearrange("p (g d) -> p g d", d=QCH)
                            nc.tensor.matmul(
                                op[0:D, 0:QCH],
                                vp8buf[h][:, kb0 * D:(kb0 + 2) * D]
                                .rearrange("p (g d) -> p g d", d=D),
                                p83,
                                start=first_pv, stop=False,
                                perf_mode=mybir.MatmulPerfMode.DoubleRow,
                            )
                            nc.tensor.matmul(
                                rsum[:, 0:QCH],
                                ones8[:].rearrange("p (g d) -> p g d", d=32),
                                p83,
                                start=first_pv, stop=(pi == npair - 1),
                                perf_mode=mybir.MatmulPerfMode.DoubleRow,
                            )
                            first_pv = False
                            after_group()
                        if npair > 0:
                            # diag blocks accumulate V rows onto the pair
                            # result (start=False); row 64 (their ones col)
                            # needs explicit zeroing first
                            nc.vector.memset(op[D:D + 1, :], 0.0)
                            # stage the pair rowsums to SBUF (DVE cannot read
                            # two PSUM operands in one op)
                            rs_sb = epi.tile([1, QCH], fp32, tag="rs")
                            nc.vector.tensor_copy(rs_sb[:], rsum[0:1, :])
                        gi = nkb - 4
                    while gi < len(blocks):
                        # greedy bank-aligned packing into [128, SCAP]
                        grp, offs = [], []
                        off = 0
                        while gi < len(blocks):
                            w = blocks[gi][2]
                            po = off
                            if po % 512 and (po % 512) + w > 512:
                                po = ((po + 511) // 512) * 512
                            if po + w > SCAP:
                                break
                            grp.append(blocks[gi])
                            offs.append(po)
                            off = po + w
                            gi += 1
                        sp = sps.tile([128, SCAP], fp32, tag="sp")
                        for (kb, qoff, w), po in zip(grp, offs):
                            nc.tensor.matmul(
                                sp[:, po:po + w],
                                qkvT[km][ko:ko + D, kb * KB:(kb + 1) * KB],
                                qkvT[qm][qo:qo + D,
                                         qc * QCH + qoff:(qc + 1) * QCH],
                                start=True, stop=True,
                            )
                        pt = ptp.tile([128, SCAP], bf16, tag="pt")
                        # coalesce contiguous spans into exp calls
                        spans = []
                        for (kb, qoff, w), po in zip(grp, offs):
                            if spans and spans[-1][1] == po:
                                spans[-1][1] = po + w
                            else:
                                spans.append([po, po + w])
                        for a, bnd in spans:
                            nc.scalar.activation(
                                pt[:, a:bnd], sp[:, a:bnd],
                                mybir.ActivationFunctionType.Exp,
                                scale=0.125, bias=ebias[0:128, :])
                        for bi, ((kb, qoff, w), po) in enumerate(zip(grp, offs)):
                            if qoff or w < QCH or kb == nkb - 4:
                                nc.vector.tensor_mul(
                                    pt[:, po:po + 128], pt[:, po:po + 128],
                                    maskt[:, :])
                            nc.tensor.matmul(
                                op[0:D + 1, qoff:QCH], vp[h][kb],
                                pt[:, po:po + w],
                                start=first_pv and qoff == 0,
                                stop=(gi >= len(blocks) and bi == len(grp) - 1),
                            )
                            if qoff == 0:
                                first_pv = False
                        after_group()
                    # normalize via gpsimd partition-broadcast of 1/rowsum
                    recip = epi.tile([1, QCH], fp32, tag="recip")
                    if use8:
                        den = epi.tile([1, QCH], fp32, tag="recip")
                        nc.vector.tensor_add(den[:], op[D:D + 1, :], rs_sb[:])
                        nc.vector.reciprocal(recip[:], den[:])
                    else:
                        nc.vector.reciprocal(recip[:], op[D:D + 1, :])
                    bcast = epi.tile([D, QCH], fp32, tag="bcast")
                    nc.gpsimd.partition_broadcast(bcast[:], recip[:], channels=D)
                    qwin = slice(qc * QCH, (qc + 1) * QCH)
                    if h < 2:
                        nc.vector.tensor_mul(
                            OT_a[h * D:(h + 1) * D, qwin], op[0:D, :], bcast[:])
                    else:
                        nc.vector.tensor_mul(
                            OT_b[:, qwin], op[0:D, :], bcast[:])
                # flush any unpaced prep units
                while uptr < len(units):
                    units[uptr]()
                    uptr += 1

              # ---- tail: last chunk's projection + RS, split in two so
              # the first half-RS overlaps the second half's matmuls ----
              do_proj(NQC - 1, half=0)
              do_proj(NQC - 1, half=1)

    nc.compile()
    return nc


def make_core_inputs(x, w_attn, w_proj, core):
    """Build the per-core input dict from full problem inputs."""
    b, hg = core // 4, core % 4
    hs = HPC * hg
    q = [w_attn[:, (hs + j) * D:(hs + j + 1) * D] for j in range(HPC)]
    k = [w_attn[:, C + (hs + j) * D:C + (hs + j + 1) * D] for j in range(HPC)]
    v = [w_attn[:, 2 * C + (hs + j) * D:2 * C + (hs + j + 1) * D] for j in range(HPC)]
    pad = np.zeros((C, D), dtype=np.float32)
    # col layout: [q0|q1, k0|k1, q2|v2, k2|pad, v0|v1]
    wqkv = np.concatenate([q[0], q[1], k[0], k[1], q[2], v[2], k[2], pad, v[0], v[1]],
                          axis=1)
    wp = w_proj[hs * D:(hs + HPC) * D, :]
    iden = np.eye(128, dtype=np.float32)
    masks = (np.arange(128)[:, None] <= np.arange(128)[None, :]).astype(np.float32)
    return {
        "x": np.ascontiguousarray(x[b]),
        "wqkv": np.ascontiguousarray(wqkv),
        "wp": np.ascontiguousarray(wp),
        "iden": iden,
        "masks": masks,
    }


_CACHE = {}


class _SpmdRunner:
    """Executes the prebuilt Bass module on the 8 axon NeuronCores via PJRT
    (mirrors concourse.bass2jax.run_bass_via_pjrt's multi-core path, but jits
    once so repeated calls are cheap)."""

    def __init__(self, nc, n_cores=N_CORES, n_iter=1, donate=True):
        import jax
        from jax.sharding import Mesh, PartitionSpec
        try:
            from jax import shard_map
            def _shard_map(f, mesh, in_specs, out_specs):
                return shard_map(f, mesh=mesh, in_specs=in_specs,
                                 out_specs=out_specs, check_vma=False)
        except ImportError:
            from jax.experimental.shard_map import shard_map
            def _shard_map(f, mesh, in_specs, out_specs):
                return shard_map(f, mesh=mesh, in_specs=in_specs,
                                 out_specs=out_specs, check_rep=False)
        import concourse.mybir as mybir
        from concourse.bass2jax import (_bass_exec_p, install_neuronx_cc_hook,
                                        partition_id_tensor)

        install_neuronx_cc_hook()
        self.nc = nc
        self.n_cores = n_cores
        partition_name = (nc.partition_id_tensor.name
                          if nc.partition_id_tensor else None)
        in_names, out_names, out_avals, zero_outs = [], [], [], []
        for alloc in nc.m.functions[0].allocations:
            if not isinstance(alloc, mybir.MemoryLocationSet):
                continue
            name = alloc.memorylocations[0].name
            if alloc.kind == "ExternalInput":
                if name != partition_name:
                    in_names.append(name)
            elif alloc.kind == "ExternalOutput":
                out_names.append(name)
                shape = tuple(alloc.tensor_shape)
                dtype = mybir.dt.np(alloc.dtype)
                out_avals.append(jax.core.ShapedArray(shape, dtype))
                zero_outs.append(np.zeros(shape, dtype))
        self.in_names, self.out_names = in_names, out_names
        self.out_avals, self.zero_outs = tuple(out_avals), zero_outs
        n_params, n_outs = len(in_names), len(out_avals)
        all_in = list(in_names) + list(out_names)
        if partition_name is not None:
            all_in.append(partition_name)

        def _body(*args):
            ins = list(args[:n_params])
            outs = list(args[n_params:])
            for _ in range(n_iter):
                operands = ins + outs
                if partition_name is not None:
                    operands.append(partition_id_tensor())
                outs = list(_bass_exec_p.bind(
                    *operands,
                    out_avals=self.out_avals,
                    in_names=tuple(all_in),
                    out_names=tuple(out_names),
                    lowering_input_output_aliases=(),
                    sim_require_finite=True,
                    sim_require_nnan=True,
                    nc=nc,
                ))
            return tuple(outs)

        devices = jax.devices()[:n_cores]
        self.mesh = Mesh(np.asarray(devices), ("core",))
        in_specs = (PartitionSpec("core"),) * (n_params + n_outs)
        out_specs = (PartitionSpec("core"),) * n_outs
        self.fn = jax.jit(
            _shard_map(_body, self.mesh, in_specs, out_specs),
            donate_argnums=(tuple(range(n_params, n_params + n_outs))
                            if donate else ()),
            keep_unused=True,
        )

    def concat_inputs(self, in_maps):
        return [
            np.concatenate([np.asarray(in_maps[c][name])
                            for c in range(self.n_cores)], axis=0)
            for name in self.in_names
        ]

    def zeros(self):
        return [np.zeros((self.n_cores * z.shape[0], *z.shape[1:]), z.dtype)
                for z in self.zero_outs]

    def __call__(self, concat_in, out_bufs=None):
        if out_bufs is None:
            out_bufs = self.zeros()
        return self.fn(*concat_in, *out_bufs)

    def split_outputs(self, out_arrs):
        res = []
        for c in range(self.n_cores):
            res.append({
                name: np.asarray(out_arrs[i]).reshape(
                    self.n_cores, *self.out_avals[i].shape)[c]
                for i, name in enumerate(self.out_names)})
        return res


def _get_runner():
    if "runner" not in _CACHE:
        nc = _build_nc()
        _CACHE["runner"] = _SpmdRunner(nc)
    return _CACHE["runner"]


def kernel(x, w_attn, w_proj):
    import jax
    x = np.asarray(x, dtype=np.float32)
    w_attn = np.asarray(w_attn, dtype=np.float32)
    w_proj = np.asarray(w_proj, dtype=np.float32)
    runner = _get_runner()
    in_maps = [make_core_inputs(x, w_attn, w_proj, c) for c in range(N_CORES)]
    ci = runner.concat_inputs(in_maps)
    r = runner(ci)
    jax.block_until_ready(r)
    res = runner.split_outputs(r)
    out = np.empty((B, T, C), dtype=np.float32)
    for c in range(N_CORES):
        b, j = c // 4, c % 4
        # chunk-c RS gives this core (group rank j) rows
        # [512*c + 128*j, 512*c + 128*(j+1)) as out rows [128c:128(c+1)];
        # the LAST chunk is reduce-scattered in two 256-row halves, so its
        # pieces are 64 rows each
        for ch in range(NQC - 1):
            out[b, 512 * ch + 128 * j:512 * ch + 128 * (j + 1), :] = \
                res[c]["out"][128 * ch:128 * (ch + 1)]
        ch = NQC - 1
        for hf in range(2):
            lo = 512 * ch + 256 * hf + 64 * j
            out[b, lo:lo + 64, :] = \
                res[c]["out"][128 * ch + 64 * hf:128 * ch + 64 * hf + 64]
    return out
